# revision 1
# baseline (speedup 1.0000x reference)
# InternLM2-7B decode-step paged attention on 8 Trainium2 NeuronCores.
#
# Sharding (tensor-parallel, per the source hooks):
#   - wqkv column-sharded: core c gets q heads 4c..4c+3 and kv head c
#   - wo row-sharded: core c gets rows for q heads 4c..4c+3
#   - KV cache sharded along the kv-head dim: core c gets head c
#   - output projection partials summed on the host (the all-reduce)
#
# Host-side prep (pure data movement / tiny math):
#   - paged-cache gather via block_offsets (a permutation of blocks),
#     head-slice, cast to bf16, and (for K) transpose to [B, HD, L]
#   - RoPE cos/sin tables from position_ids_1d
#   - additive validity mask from kv_seqlens
#
# Device math per core (see _emit):
#   qkv = hT.T @ wqkv_shard; rope(q, k); scores = (q*scale)_bf16 @ kT_bf16
#   with the new token's score computed separately in f32; masked batched
#   softmax over [128 rows = 32 seqs x 4 heads, 4097]; out = probs @ v + p_new*v_new;
#   y_partial = out @ wo_shard (f32).
import os
import sys

for _p in (
    "/opt/trn_rl_repo",
    "/root/.axon_site",
    "/root/.axon_site/_ro/trn_rl_repo",
    "/root/.axon_site/_ro/pypackages",
):
    if os.path.isdir(_p) and _p not in sys.path:
        sys.path.append(_p)

import numpy as np
import ml_dtypes

BF16NP = ml_dtypes.bfloat16

import concourse.bass as bass
from concourse import bacc
import concourse.mybir as mybir
import concourse.tile as tile
from concourse.masks import make_identity

B = 32          # batch (decoding sequences)
H = 32          # query heads
KVH = 8         # kv heads
G = 4           # query heads per kv head (= per core)
HD = 128        # head dim
D = 4096        # model dim
W = (G + 2) * HD  # per-core qkv shard width = 768
L = 4096        # kv positions per sequence
BLOCK = 64
NBLK = 64
NCORES = 8
THETA = 1e6

F32 = mybir.dt.float32
BF16 = mybir.dt.bfloat16
SCALE = 1.0 / float(np.sqrt(HD))
NEG = -1.0e30


def _emit(nc, tc, hT, wq, wo, kT, vv, cs, mk, y, B_, L_):
    """Emit the per-core program (v2: few big DMAs, stage DMAs on SWDGE).

    All SBUF compute APs start at partition 0; partition scatter is done
    with SBUF<->SBUF DMAs (exempt from the 32-strip start-partition rule).
    """
    import contextlib

    R = G * B_            # score rows (seq-major: row = s*G + h)
    NT = L_ // 512        # kT 512-col chunks
    NA = L_ // 128        # 128-pos tiles
    KT_ = D // 128        # contraction tiles for the qkv projection
    X = mybir.AxisListType.X

    with contextlib.ExitStack() as ctx:
        singles = ctx.enter_context(tc.tile_pool(name="singles", bufs=1))
        wqp = ctx.enter_context(tc.tile_pool(name="wqp", bufs=2))
        ktp = ctx.enter_context(tc.tile_pool(name="ktp", bufs=3))
        vtp = ctx.enter_context(tc.tile_pool(name="vtp", bufs=3))
        stg = ctx.enter_context(tc.tile_pool(name="stg", bufs=3))
        # PSUM budget (8 banks): qy 2x1 + scs 2x2 + po 1 + tr 1
        psp = ctx.enter_context(tc.tile_pool(name="psp", bufs=2, space="PSUM"))
        psp1 = ctx.enter_context(tc.tile_pool(name="psp1", bufs=1, space="PSUM"))

        ident = singles.tile([128, 128], F32)
        make_identity(nc, ident)
        ident_bf = singles.tile([128, 128], BF16)
        make_identity(nc, ident_bf)

        hT_sb = singles.tile([128, KT_, B_], BF16)
        nc.sync.dma_start(hT_sb, hT.rearrange("(t p) b -> p t b", p=128))
        cs_sb = singles.tile([B_, HD], F32)
        nc.sync.dma_start(cs_sb, cs)
        mask_sb = singles.tile([R, L_ + 1], F32)
        nc.sync.dma_start(mask_sb, mk)
        wo_sb = singles.tile([128, G, D], BF16)
        nc.sync.dma_start(wo_sb, wo.rearrange("(h p) n -> p h n", p=128))

        # ---- fused QKV projection: qkv[B_, W] = hT.T @ wq ----
        ps_q0 = psp.tile([128, 384], F32, tag="qy")
        ps_q1 = psp.tile([128, 384], F32, tag="qy")
        wqv = wq.rearrange("(t p) w -> p t w", p=128)
        for tq in range(KT_ // 4):
            wt = wqp.tile([128, 4, W], BF16, tag="wt")
            nc.sync.dma_start(wt, wqv[:, tq * 4 : (tq + 1) * 4, :])
            for u in range(4):
                t = tq * 4 + u
                nc.tensor.matmul(ps_q0[:B_, :], lhsT=hT_sb[:, t, :],
                                 rhs=wt[:, u, 0:384],
                                 start=(t == 0), stop=(t == KT_ - 1))
                nc.tensor.matmul(ps_q1[:B_, :], lhsT=hT_sb[:, t, :],
                                 rhs=wt[:, u, 384:W],
                                 start=(t == 0), stop=(t == KT_ - 1))
        qkv_sb = singles.tile([B_, W], F32)
        nc.vector.tensor_copy(qkv_sb[:, 0:384], ps_q0[:B_, :])
        nc.vector.tensor_copy(qkv_sb[:, 384:W], ps_q1[:B_, :])

        # ---- RoPE on q (G heads) and k (1 head); v passthrough ----
        q_sb = singles.tile([B_, G * HD], F32)
        k_sb = singles.tile([B_, HD], F32)
        v_sb = singles.tile([B_, HD], F32)
        nc.vector.tensor_copy(v_sb, qkv_sb[:, (G + 1) * HD : (G + 2) * HD])
        cosv = cs_sb[:, 0:64]
        sinv = cs_sb[:, 64:128]
        for j in range(G + 1):
            src = qkv_sb[:, j * HD : (j + 1) * HD]
            dst = q_sb[:, j * HD : (j + 1) * HD] if j < G else k_sb[:, :]
            a = src[:, 0:64]
            b = src[:, 64:128]
            t1 = stg.tile([B_, 64], F32, tag="rt1")
            t2 = stg.tile([B_, 64], F32, tag="rt2")
            nc.vector.tensor_mul(t1, a, cosv)
            nc.vector.tensor_mul(t2, b, sinv)
            nc.vector.tensor_sub(dst[:, 0:64], t1, t2)
            t3 = stg.tile([B_, 64], F32, tag="rt1")
            t4 = stg.tile([B_, 64], F32, tag="rt2")
            nc.vector.tensor_mul(t3, b, cosv)
            nc.vector.tensor_mul(t4, a, sinv)
            nc.vector.tensor_add(dst[:, 64:128], t3, t4)

        # ---- qT (pre-scaled, bf16): qT_buf[d, s, h] ----
        qT_buf = singles.tile([128, B_, G], BF16)
        for h in range(G):
            ps_t = psp1.tile([128, 128], F32, tag="tr")
            nc.tensor.transpose(ps_t[:, :B_], q_sb[:, h * HD : (h + 1) * HD],
                                ident[:B_, :B_])
            nc.vector.tensor_scalar_mul(out=qT_buf[:, :, h], in0=ps_t[:, :B_],
                                        scalar1=SCALE)

        # ---- new-token score (f32): row-major copies via DMA reshape ----
        q_row = singles.tile([R, HD], F32)
        nc.gpsimd.dma_start(q_row, q_sb[:, :])
        k_rep = singles.tile([R, HD], F32)
        nc.gpsimd.dma_start(k_rep, k_sb[:, None, :].to_broadcast((B_, G, HD)))
        v_rep = singles.tile([R, HD], F32)
        nc.gpsimd.dma_start(v_rep, v_sb[:, None, :].to_broadcast((B_, G, HD)))

        scores = singles.tile([R, L_ + 1], F32)
        tsn = singles.tile([R, HD], F32)
        nc.vector.tensor_mul(tsn, q_row, k_rep)
        nc.vector.reduce_sum(out=scores[:, L_ : L_ + 1], in_=tsn, axis=X)
        nc.scalar.mul(scores[:, L_ : L_ + 1], scores[:, L_ : L_ + 1], SCALE)

        # ---- softmax state (filled per seq-group) ----
        mx = singles.tile([R, 1], F32)
        ngm = singles.tile([R, 1], F32)
        sm = singles.tile([R, 1], F32)
        rc = singles.tile([R, 1], F32)
        pnew = singles.tile([R, 1], F32)
        probs = singles.tile([R, L_ + 1], BF16)
        attnT = singles.tile([128, NA, R], BF16)
        out_all = singles.tile([R, HD], F32)

        # ---- pipelined over groups of 8 seqs (32 rows, legal strip bases):
        #      scores(g) -> softmax(g) -> attnT(g) -> V(g), overlapped by Tile
        SG = 8 if B_ % 8 == 0 else B_
        RG = G * SG
        for grp in range(B_ // SG):
            rows = slice(grp * RG, (grp + 1) * RG)
            # cache scores: one kT DMA per seq, 1024-wide psum slabs
            for s in range(grp * SG, (grp + 1) * SG):
                kt_t = ktp.tile([128, L_], BF16, tag="kt")
                nc.sync.dma_start(kt_t, kT[s, :, :])
                for half in range(NT // 2):
                    ps_sc = psp.tile([128, 1024], F32, tag="scs")
                    for u in range(2):
                        tg = half * 2 + u
                        nc.tensor.matmul(ps_sc[:G, u * 512 : (u + 1) * 512],
                                         lhsT=qT_buf[:, s, :],
                                         rhs=kt_t[:, tg * 512 : (tg + 1) * 512],
                                         start=True, stop=True)
                    sct = stg.tile([G, 1024], F32, tag="sct")
                    nc.any.tensor_copy(sct, ps_sc[:G, :])
                    nc.gpsimd.dma_start(
                        scores[s * G : (s + 1) * G,
                               half * 1024 : (half + 1) * 1024], sct)

            # masked softmax over this group's rows
            nc.vector.tensor_add(scores[rows, :], scores[rows, :],
                                 mask_sb[rows, :])
            nc.vector.reduce_max(out=mx[rows, :], in_=scores[rows, :], axis=X)
            nc.scalar.mul(ngm[rows, :], mx[rows, :], -1.0)
            nc.scalar.activation(out=probs[rows, :], in_=scores[rows, :],
                                 func=mybir.ActivationFunctionType.Exp,
                                 bias=ngm[rows, :], scale=1.0,
                                 accum_out=sm[rows, :])
            nc.scalar.activation(out=pnew[rows, :],
                                 in_=scores[rows, L_ : L_ + 1],
                                 func=mybir.ActivationFunctionType.Exp,
                                 bias=ngm[rows, :], scale=1.0)
            nc.vector.reciprocal(rc[rows, :], sm[rows, :])

            # transpose this group's probs into attnT[pos, row]
            for t in range(NA):
                ps_t = psp.tile([128, 128], BF16, tag="qy")
                nc.tensor.transpose(ps_t[:, :RG],
                                    probs[rows, t * 128 : (t + 1) * 128],
                                    ident_bf[rows, rows],
                                    tile_position=(grp * RG % 128, 0))
                nc.any.tensor_copy(attnT[:, t, rows], ps_t[:, :RG])

            # V accumulation: one vv DMA per seq
            for s in range(grp * SG, (grp + 1) * SG):
                vt = vtp.tile([128, NA, HD], BF16, tag="vt")
                nc.sync.dma_start(vt, vv[s, :, :, :])
                ps_o = psp1.tile([128, HD], F32, tag="po")
                for ti in range(NA):
                    nc.tensor.matmul(ps_o[:G, :],
                                     lhsT=attnT[:, ti, s * G : (s + 1) * G],
                                     rhs=vt[:, ti, :],
                                     start=(ti == 0), stop=(ti == NA - 1))
                ost = stg.tile([G, HD], F32, tag="ost")
                nc.any.tensor_copy(ost, ps_o[:G, :])
                nc.gpsimd.dma_start(out_all[s * G : (s + 1) * G, :], ost)

        # normalize + new-token contribution
        nc.vector.tensor_scalar_mul(out=out_all, in0=out_all, scalar1=rc)
        pn2 = singles.tile([R, 1], F32)
        tvn = singles.tile([R, HD], F32)
        nc.vector.tensor_mul(pn2, pnew, rc)
        nc.vector.tensor_scalar_mul(out=tvn, in0=v_rep, scalar1=pn2)
        nc.vector.tensor_add(out_all, out_all, tvn)

        # ---- transpose out_all -> outT[d, h, s] (bf16 for the wo matmul) ----
        ps_ot = psp1.tile([128, 128], F32, tag="tr")
        nc.tensor.transpose(ps_ot[:, :R], out_all, ident[:R, :R])
        outT = singles.tile([128, G, B_], BF16)
        nc.vector.tensor_copy(outT.rearrange("p h s -> p s h"),
                              ps_ot[:, :R].rearrange("p (s h) -> p s h", h=G))

        # ---- output projection partial: y = outT.T @ wo_shard ----
        for n in range(D // 512):
            ps_y = psp.tile([128, 512], F32, tag="qy")
            for h in range(G):
                nc.tensor.matmul(ps_y[:B_, :], lhsT=outT[:, h, :],
                                 rhs=wo_sb[:, h, n * 512 : (n + 1) * 512],
                                 start=(h == 0), stop=(h == G - 1))
            yst = stg.tile([B_, 512], F32, tag="yst")
            nc.any.tensor_copy(yst, ps_y[:B_, :])
            nc.sync.dma_start(y[:, n * 512 : (n + 1) * 512], yst)


_NC_CACHE = None


def build_bass():
    global _NC_CACHE
    if _NC_CACHE is not None:
        return _NC_CACHE
    nc = bacc.Bacc("TRN2")
    hT = nc.dram_tensor("hT", [D, B], BF16, kind="ExternalInput")
    wq = nc.dram_tensor("wq", [D, W], BF16, kind="ExternalInput")
    wo = nc.dram_tensor("wo", [G * HD, D], BF16, kind="ExternalInput")
    kT = nc.dram_tensor("kT", [B, HD, L], BF16, kind="ExternalInput")
    vv = nc.dram_tensor("vv", [B, 128, L // 128, HD], BF16, kind="ExternalInput")
    cs = nc.dram_tensor("cs", [B, HD], F32, kind="ExternalInput")
    mk = nc.dram_tensor("mk", [G * B, L + 1], F32, kind="ExternalInput")
    y = nc.dram_tensor("y", [B, D], F32, kind="ExternalOutput")
    with tile.TileContext(nc) as tc:
        _emit(nc, tc, hT[:, :], wq[:, :], wo[:, :], kT[:, :, :], vv[:, :, :],
              cs[:, :], mk[:, :], y[:, :], B, L)
    nc.finalize()  # runs Bacc.compile(): wait legalization, reg alloc, DCE
    _NC_CACHE = nc
    return nc


def make_host_inputs(hidden_states, wqkv, wo, k_cache, v_cache,
                     position_ids_1d, block_offsets, kv_seqlens):
    """Shard + preprocess full inputs into 8 per-core in_maps."""
    hidden_states = np.asarray(hidden_states, dtype=np.float32)
    wqkv = np.asarray(wqkv, dtype=np.float32)
    wo = np.asarray(wo, dtype=np.float32)
    k_cache = np.asarray(k_cache, dtype=np.float32)
    v_cache = np.asarray(v_cache, dtype=np.float32)
    position_ids_1d = np.asarray(position_ids_1d, dtype=np.int32)
    block_offsets = np.asarray(block_offsets, dtype=np.int32)
    kv_seqlens = np.asarray(kv_seqlens, dtype=np.int32)

    hT = np.ascontiguousarray(hidden_states.T).astype(BF16NP)  # [D, B]

    # RoPE tables (f32, matching the reference convention)
    inv_freq = (1.0 / (THETA ** (np.arange(0, HD, 2, dtype=np.float64) / HD)))
    ang = position_ids_1d.astype(np.float64)[:, None] * inv_freq[None, :]
    cs_host = np.concatenate(
        [np.cos(ang), np.sin(ang)], axis=1).astype(np.float32)  # [B, 128]

    # additive mask over [rows = s*G+h, L+1]; cache col j valid iff
    # j < seqlen-1 (the cache row at seqlen-1 is replaced by the new token,
    # which lives in the extra column L and is always valid)
    j = np.arange(L, dtype=np.int64)[None, :]
    valid = j < (kv_seqlens.astype(np.int64)[:, None] - 1)
    mask_seq = np.where(valid, 0.0, NEG).astype(np.float32)  # [B, L]
    mask_seq = np.concatenate(
        [mask_seq, np.zeros((B, 1), np.float32)], axis=1)  # [B, L+1]
    mask = np.repeat(mask_seq, G, axis=0)  # [G*B, L+1]

    # paged gather: per-sequence kv via block table (a permutation of blocks)
    ident_blocks = np.array_equal(block_offsets.ravel(),
                                  np.arange(B * NBLK, dtype=np.int64))

    kx = np.moveaxis(k_cache, 2, 0)  # [KVH, NUM_BLOCKS, BLOCK, HD] (view)
    vx = np.moveaxis(v_cache, 2, 0)

    in_maps = []
    for c in range(NCORES):
        if ident_blocks:
            kg = kx[c].reshape(B, L, HD)
            vg = vx[c].reshape(B, L, HD)
        else:
            kg = kx[c][block_offsets].reshape(B, L, HD)
            vg = vx[c][block_offsets].reshape(B, L, HD)
        kT_c = np.ascontiguousarray(
            kg.astype(BF16NP).transpose(0, 2, 1))          # [B, HD, L]
        # swizzle so each SBUF partition's data is contiguous in DRAM:
        # v_sw[s, p, a, d] = v[s, a*128+p, d]
        v_c = np.ascontiguousarray(
            vg.astype(BF16NP).reshape(B, L // 128, 128, HD).transpose(0, 2, 1, 3))
        wq_c = np.ascontiguousarray(np.concatenate([
            wqkv[:, c * G * HD : (c + 1) * G * HD],
            wqkv[:, H * HD + c * HD : H * HD + (c + 1) * HD],
            wqkv[:, (H + KVH) * HD + c * HD : (H + KVH) * HD + (c + 1) * HD],
        ], axis=1)).astype(BF16NP)                         # [D, W]
        wo_c = np.ascontiguousarray(
            wo[c * G * HD : (c + 1) * G * HD, :]).astype(BF16NP)  # [G*HD, D]
        in_maps.append(dict(hT=hT, wq=wq_c, wo=wo_c, kT=kT_c, vv=v_c,
                            cs=cs_host, mk=mask))
    return in_maps


def kernel(**inputs):
    from concourse.bass_utils import run_bass_kernel_spmd

    in_maps = make_host_inputs(
        inputs["hidden_states"], inputs["wqkv"], inputs["wo"],
        inputs["k_cache"], inputs["v_cache"], inputs["position_ids_1d"],
        inputs["block_offsets"], inputs["kv_seqlens"])
    nc = build_bass()
    res = run_bass_kernel_spmd(nc, in_maps, core_ids=list(range(NCORES)))
    y = np.zeros((B, D), dtype=np.float32)
    for r in res.results:
        y += np.asarray(r["y"], dtype=np.float32)
    return y



# revision 2
# speedup vs baseline: 1.0811x; 1.0811x over previous
# InternLM2-7B decode-step paged attention on 8 Trainium2 NeuronCores, v2.
#
# Sharding (tensor-parallel, per the source hooks):
#   - wqkv column-sharded: core c gets q heads 4c..4c+3 and kv head c
#   - wo row-sharded: core c gets rows for q heads 4c..4c+3
#   - KV cache sharded along the kv-head dim: core c gets head c
#   - output projection partials summed on the host (the all-reduce)
#
# v2 design (vs the v1 row-major kernel):
#   - scores computed directly TRANSPOSED: per (seq, l-chunk) the K-tile
#     [d=128, l=128] is the PE stationary operand and q [d, 4 heads] the
#     moving one, so psum holds S^T[l, (s,h)] with no row-scatter DMAs.
#   - softmax without max-subtraction (scores here are bounded ~|s|<=10,
#     exp stays in f32 range; softmax is shift-invariant) -> exp straight
#     from psum into bf16 attnT, already in the layout the V matmul needs.
#     A multiplicative 0/1 mask (mz) zeroes invalid cache positions.
#   - denominators via ones-vector stationary matmuls over attnT chunks;
#     normalization by a rank-1 replicate matmul of 1/sums, one DVE mul
#     per seq.
#   - V accumulated transposed too: V-chunk [l=128, d=128] stationary,
#     attn [l, 4] moving -> psum [d, 4] per seq; no output transposes.
#   - new token handled as a rank-1 (K=1) outer-product matmul appended
#     to each seq's V accumulation group.
import os
import sys

for _p in (
    "/opt/trn_rl_repo",
    "/root/.axon_site",
    "/root/.axon_site/_ro/trn_rl_repo",
    "/root/.axon_site/_ro/pypackages",
):
    if os.path.isdir(_p) and _p not in sys.path:
        sys.path.append(_p)

import numpy as np
import ml_dtypes

BF16NP = ml_dtypes.bfloat16

import concourse.bass as bass
from concourse import bacc
import concourse.mybir as mybir
import concourse.tile as tile
from concourse.masks import make_identity

B = 32          # batch (decoding sequences)
H = 32          # query heads
KVH = 8         # kv heads
G = 4           # query heads per kv head (= per core)
HD = 128        # head dim
D = 4096        # model dim
W = (G + 2) * HD  # per-core qkv shard width = 768
L = 4096        # kv positions per sequence
NCH = L // 128  # 32 l-chunks of 128
CGK = 2         # l-chunks per kT DMA tile / psum slab
CGN = NCH // CGK  # 16 chunk groups
VSG = 2         # seqs per v DMA tile
KT_ = D // 128  # 32 contraction tiles for the qkv projection
BLOCK = 64
NBLK = 64
NCORES = 8
THETA = 1e6
R = G * B       # 128 row-cols (s-major: col = 4*s + h)

F32 = mybir.dt.float32
BF16 = mybir.dt.bfloat16
SCALE = 1.0 / float(np.sqrt(HD))


def _emit(nc, tc, hT, wq, wo, kTg, vv, mz, cs, y):
    import contextlib

    Exp = mybir.ActivationFunctionType.Exp

    with contextlib.ExitStack() as ctx:
        singles = ctx.enter_context(tc.tile_pool(name="singles", bufs=1))
        wqp = ctx.enter_context(tc.tile_pool(name="wqp", bufs=2))
        ktp = ctx.enter_context(tc.tile_pool(name="ktp", bufs=4))
        vtp = ctx.enter_context(tc.tile_pool(name="vtp", bufs=3))
        stg = ctx.enter_context(tc.tile_pool(name="stg", bufs=3))
        # PSUM (8 banks): scp 3x1 + po 3x1 + sums 1 + rcps 1
        psA = ctx.enter_context(tc.tile_pool(name="psA", bufs=3, space="PSUM"))
        psB = ctx.enter_context(tc.tile_pool(name="psB", bufs=3, space="PSUM"))
        psD = ctx.enter_context(tc.tile_pool(name="psD", bufs=1, space="PSUM"))

        ident = singles.tile([128, 128], F32)
        make_identity(nc, ident)

        # ---- input loads (sync ring: hT/cs/wq/vv/y; scalar ring: mz/wo/kT)
        hT_sb = singles.tile([128, KT_, B], BF16)
        nc.sync.dma_start(hT_sb, hT)
        cs_sb = singles.tile([B, HD], F32)
        nc.sync.dma_start(cs_sb, cs)
        mz_sb = singles.tile([128, NCH, R], BF16)
        nc.scalar.dma_start(mz_sb, mz)
        wo_sb = singles.tile([128, G, D], BF16)
        nc.scalar.dma_start(wo_sb, wo.rearrange("(h p) n -> p h n", p=128))

        attnT = singles.tile([128, NCH, R], BF16)
        qT_buf = singles.tile([128, B, G], BF16)
        k_newT = singles.tile([128, B], F32)
        tmp_kq = singles.tile([128, B, G], F32)
        ones_bf = singles.tile([128, 1], BF16)
        nc.vector.memset(ones_bf, 1.0)
        ones_f = singles.tile([128, 1], F32)
        nc.vector.memset(ones_f, 1.0)
        ones_row = singles.tile([1, 128], F32)
        nc.vector.memset(ones_row, 1.0)
        pnew_row = singles.tile([1, R], BF16)
        vnew_row = singles.tile([1, B, HD], BF16)
        sums_f = singles.tile([1, R], F32)
        rc_row = singles.tile([1, R], F32)
        rc_sb = singles.tile([128, R], F32)
        outT_bf = singles.tile([128, B, G], BF16)

        # ---- fused QKV projection: qkv[B, W] = hT.T @ wq ----
        ps_q0 = psA.tile([128, 512], F32, tag="scp")
        ps_q1 = psA.tile([128, 512], F32, tag="scp")
        wqv = wq.rearrange("(t p) w -> p t w", p=128)
        for tq in range(KT_ // 4):
            wt = wqp.tile([128, 4, W], BF16, tag="wt")
            nc.sync.dma_start(wt, wqv[:, tq * 4 : (tq + 1) * 4, :])
            for u in range(4):
                t = tq * 4 + u
                nc.tensor.matmul(ps_q0[:B, 0:384], lhsT=hT_sb[:, t, :],
                                 rhs=wt[:, u, 0:384],
                                 start=(t == 0), stop=(t == KT_ - 1))
                nc.tensor.matmul(ps_q1[:B, 0:384], lhsT=hT_sb[:, t, :],
                                 rhs=wt[:, u, 384:W],
                                 start=(t == 0), stop=(t == KT_ - 1))
        qkv_sb = singles.tile([B, W], F32)
        nc.vector.tensor_copy(qkv_sb[:, 0:384], ps_q0[:B, 0:384])
        nc.vector.tensor_copy(qkv_sb[:, 384:W], ps_q1[:B, 0:384])

        # ---- RoPE on q (G heads) and k (1 head); v passthrough ----
        q_sb = singles.tile([B, G * HD], F32)
        k_sb = singles.tile([B, HD], F32)
        v_sb = singles.tile([B, HD], F32)
        nc.vector.tensor_copy(v_sb, qkv_sb[:, (G + 1) * HD : (G + 2) * HD])
        cosv = cs_sb[:, 0:64]
        sinv = cs_sb[:, 64:128]
        for j in range(G + 1):
            src = qkv_sb[:, j * HD : (j + 1) * HD]
            dst = q_sb[:, j * HD : (j + 1) * HD] if j < G else k_sb[:, :]
            a = src[:, 0:64]
            b = src[:, 64:128]
            t1 = stg.tile([B, 64], F32, tag="rt1")
            t2 = stg.tile([B, 64], F32, tag="rt2")
            nc.vector.tensor_mul(t1, a, cosv)
            nc.vector.tensor_mul(t2, b, sinv)
            nc.vector.tensor_sub(dst[:, 0:64], t1, t2)
            t3 = stg.tile([B, 64], F32, tag="rt1")
            t4 = stg.tile([B, 64], F32, tag="rt2")
            nc.vector.tensor_mul(t3, b, cosv)
            nc.vector.tensor_mul(t4, a, sinv)
            nc.vector.tensor_add(dst[:, 64:128], t3, t4)

        # ---- qT (pre-scaled, bf16): qT_buf[d, s, h]; k_newT[d, s] ----
        for h in range(G):
            ps_t = psA.tile([128, 512], F32, tag="scp")
            nc.tensor.transpose(ps_t[:, :B], q_sb[:, h * HD : (h + 1) * HD],
                                ident[:B, :B])
            nc.vector.tensor_scalar_mul(out=qT_buf[:, :, h], in0=ps_t[:, :B],
                                        scalar1=SCALE)
        ps_t = psA.tile([128, 512], F32, tag="scp")
        nc.tensor.transpose(ps_t[:, :B], k_sb[:, :], ident[:B, :B])
        nc.vector.tensor_copy(k_newT, ps_t[:, :B])

        # ---- new-token: p_new[1, (s,h)] = exp(qT . k_new); v_new row ----
        v_sbb = singles.tile([B, HD], BF16)
        nc.vector.tensor_copy(v_sbb, v_sb)
        nc.gpsimd.dma_start(vnew_row[0:1, :, :], v_sbb[:, :])
        for s in range(B):
            nc.vector.tensor_scalar_mul(out=tmp_kq[:, s, :],
                                        in0=qT_buf[:, s, :],
                                        scalar1=k_newT[:, s : s + 1])
        pnew_ps = psB.tile([128, 128], F32, tag="po")
        nc.tensor.matmul(pnew_ps[0:1, 0:R], lhsT=ones_f[:, 0:1],
                         rhs=tmp_kq[:, :, :], start=True, stop=True)
        nc.scalar.activation(out=pnew_row[0:1, :], in_=pnew_ps[0:1, 0:R],
                             func=Exp)

        # ---- scores: attnT[l, c, (s,h)] = exp(S^T) * mask; sums ----
        sums_ps = psD.tile([1, R], F32, tag="sums")
        for cg in range(CGN):
            kt = ktp.tile([128, B, CGK, 128], BF16, tag="kt")
            nc.sync.dma_start(kt, kTg[cg, :, :, :, :])
            scp = psA.tile([128, 512], F32, tag="scp")
            for s in range(B):
                for u in range(CGK):
                    o = u * 128 + 4 * s
                    nc.tensor.matmul(scp[:, o : o + 4], lhsT=kt[:, s, u, :],
                                     rhs=qT_buf[:, s, :],
                                     start=True, stop=True)
            att = attnT[:, CGK * cg : CGK * cg + CGK, :]
            nc.scalar.activation(out=att.rearrange("p u r -> p (u r)"),
                                 in_=scp[:, 0 : CGK * 128], func=Exp)
            nc.vector.tensor_mul(att, att,
                                 mz_sb[:, CGK * cg : CGK * cg + CGK, :])
            for u in range(CGK):
                c = CGK * cg + u
                nc.tensor.matmul(sums_ps[0:1, :], lhsT=ones_bf[:, 0:1],
                                 rhs=attnT[:, c, :],
                                 start=(c == 0), stop=(c == NCH - 1))

        # ---- 1/(sums + p_new), replicated to all partitions ----
        nc.vector.tensor_copy(sums_f, sums_ps[0:1, :])
        nc.vector.tensor_add(sums_f, sums_f, pnew_row[0:1, :])
        nc.vector.reciprocal(rc_row, sums_f)
        rc_ps = psD.tile([128, R], F32, tag="rcps")
        nc.tensor.matmul(rc_ps[:, :], lhsT=ones_row[0:1, :],
                         rhs=rc_row[0:1, :], start=True, stop=True)
        nc.vector.tensor_copy(rc_sb, rc_ps[:, :])

        # ---- V phase: outT[d, (s,h)] = (V^T @ attn + v_new x p_new) * rc
        for sg in range(B // VSG):
            vt = vtp.tile([128, VSG, NCH, HD], BF16, tag="vt")
            nc.sync.dma_start(
                vt, vv[sg * VSG : (sg + 1) * VSG, :, :, :].rearrange(
                    "s p c d -> p s c d"))
            for sl in range(VSG):
                s = sg * VSG + sl
                ps_o = psB.tile([128, 128], F32, tag="po")
                for c in range(NCH):
                    nc.tensor.matmul(ps_o[:, 0:4], lhsT=vt[:, sl, c, :],
                                     rhs=attnT[:, c, 4 * s : 4 * s + 4],
                                     start=(c == 0), stop=False)
                nc.tensor.matmul(ps_o[:, 0:4], lhsT=vnew_row[0:1, s, :],
                                 rhs=pnew_row[0:1, 4 * s : 4 * s + 4],
                                 start=False, stop=True)
                nc.vector.tensor_mul(outT_bf[:, s, :], ps_o[:, 0:4],
                                     rc_sb[:, 4 * s : 4 * s + 4])

        # ---- output projection partial: y = outT.T @ wo_shard ----
        for n in range(D // 512):
            ps_y = psA.tile([128, 512], F32, tag="scp")
            for h in range(G):
                nc.tensor.matmul(ps_y[:B, :], lhsT=outT_bf[:, :, h],
                                 rhs=wo_sb[:, h, n * 512 : (n + 1) * 512],
                                 start=(h == 0), stop=(h == G - 1))
            yst = stg.tile([B, 512], F32, tag="yst")
            nc.any.tensor_copy(yst, ps_y[:B, :])
            nc.scalar.dma_start(y[:, n * 512 : (n + 1) * 512], yst)


_NC_CACHE = None


def build_bass():
    global _NC_CACHE
    if _NC_CACHE is not None:
        return _NC_CACHE
    nc = bacc.Bacc("TRN2")
    hT = nc.dram_tensor("hT", [128, KT_, B], BF16, kind="ExternalInput")
    wq = nc.dram_tensor("wq", [D, W], BF16, kind="ExternalInput")
    wo = nc.dram_tensor("wo", [G * HD, D], BF16, kind="ExternalInput")
    kTg = nc.dram_tensor("kTg", [CGN, 128, B, CGK, 128], BF16,
                         kind="ExternalInput")
    vv = nc.dram_tensor("vv", [B, 128, NCH, HD], BF16, kind="ExternalInput")
    mz = nc.dram_tensor("mz", [128, NCH, R], BF16, kind="ExternalInput")
    cs = nc.dram_tensor("cs", [B, HD], F32, kind="ExternalInput")
    y = nc.dram_tensor("y", [B, D], F32, kind="ExternalOutput")
    with tile.TileContext(nc) as tc:
        _emit(nc, tc, hT[:, :, :], wq[:, :], wo[:, :], kTg[:, :, :, :, :],
              vv[:, :, :, :], mz[:, :, :], cs[:, :], y[:, :])
    nc.finalize()
    _NC_CACHE = nc
    return nc


def make_host_inputs(hidden_states, wqkv, wo, k_cache, v_cache,
                     position_ids_1d, block_offsets, kv_seqlens):
    """Shard + preprocess full inputs into 8 per-core in_maps."""
    hidden_states = np.asarray(hidden_states, dtype=np.float32)
    wqkv = np.asarray(wqkv, dtype=np.float32)
    wo = np.asarray(wo, dtype=np.float32)
    k_cache = np.asarray(k_cache, dtype=np.float32)
    v_cache = np.asarray(v_cache, dtype=np.float32)
    position_ids_1d = np.asarray(position_ids_1d, dtype=np.int32)
    block_offsets = np.asarray(block_offsets, dtype=np.int32)
    kv_seqlens = np.asarray(kv_seqlens, dtype=np.int32)

    hTd = np.ascontiguousarray(
        hidden_states.T.reshape(KT_, 128, B).transpose(1, 0, 2)
    ).astype(BF16NP)  # [128, KT_, B]

    # RoPE tables (f32, matching the reference convention)
    inv_freq = (1.0 / (THETA ** (np.arange(0, HD, 2, dtype=np.float64) / HD)))
    ang = position_ids_1d.astype(np.float64)[:, None] * inv_freq[None, :]
    cs_host = np.concatenate(
        [np.cos(ang), np.sin(ang)], axis=1).astype(np.float32)  # [B, 128]

    # validity: cache position j valid iff j < seqlen-1 (cache row at
    # seqlen-1 is replaced by the new token, handled separately)
    j = np.arange(L, dtype=np.int64)[None, :]
    valid = (j < (kv_seqlens.astype(np.int64)[:, None] - 1))  # [B, L] bool

    # multiplicative bf16 mask in attnT layout [p, c, 4s+h]
    validT = valid.reshape(B, NCH, 128).transpose(2, 1, 0)  # [p, c, s]
    mz_host = np.ascontiguousarray(
        np.repeat(validT.astype(np.float32), G, axis=2)).astype(BF16NP)

    # paged gather: per-sequence kv via block table (a permutation of blocks)
    ident_blocks = np.array_equal(block_offsets.ravel(),
                                  np.arange(B * NBLK, dtype=np.int64))

    kx = np.moveaxis(k_cache, 2, 0)  # [KVH, NUM_BLOCKS, BLOCK, HD] (view)
    vx = np.moveaxis(v_cache, 2, 0)

    vmaskf = valid.astype(np.float32)[:, :, None]  # [B, L, 1]

    in_maps = []
    for c in range(NCORES):
        if ident_blocks:
            kg = kx[c].reshape(B, L, HD)
            vg = vx[c].reshape(B, L, HD)
        else:
            kg = kx[c][block_offsets].reshape(B, L, HD)
            vg = vx[c][block_offsets].reshape(B, L, HD)
        # kTg[cg, d, s, u, l] = K[s, (CGK*cg+u)*128+l, d]
        kTg_c = np.ascontiguousarray(
            kg.reshape(B, CGN, CGK, 128, HD).transpose(1, 4, 0, 2, 3)
        ).astype(BF16NP)
        # vv[s, p, c, d] = V[s, c*128+p, d], invalid positions zeroed
        vm = vg * vmaskf
        vv_c = np.ascontiguousarray(
            vm.reshape(B, NCH, 128, HD).transpose(0, 2, 1, 3)).astype(BF16NP)
        wq_c = np.ascontiguousarray(np.concatenate([
            wqkv[:, c * G * HD : (c + 1) * G * HD],
            wqkv[:, H * HD + c * HD : H * HD + (c + 1) * HD],
            wqkv[:, (H + KVH) * HD + c * HD : (H + KVH) * HD + (c + 1) * HD],
        ], axis=1)).astype(BF16NP)                         # [D, W]
        wo_c = np.ascontiguousarray(
            wo[c * G * HD : (c + 1) * G * HD, :]).astype(BF16NP)  # [G*HD, D]
        in_maps.append(dict(hT=hTd, wq=wq_c, wo=wo_c, kTg=kTg_c, vv=vv_c,
                            mz=mz_host, cs=cs_host))
    return in_maps


def kernel(**inputs):
    from concourse.bass_utils import run_bass_kernel_spmd

    in_maps = make_host_inputs(
        inputs["hidden_states"], inputs["wqkv"], inputs["wo"],
        inputs["k_cache"], inputs["v_cache"], inputs["position_ids_1d"],
        inputs["block_offsets"], inputs["kv_seqlens"])
    nc = build_bass()
    res = run_bass_kernel_spmd(nc, in_maps, core_ids=list(range(NCORES)))
    y = np.zeros((B, D), dtype=np.float32)
    for r in res.results:
        y += np.asarray(r["y"], dtype=np.float32)
    return y
